# revision 19
# baseline (speedup 1.0000x reference)
"""Trainium2 Bass kernel for nn_FragAttention (segment_reduce).

Reference computation (S=128, B=512, D=512, G=S-1=127):
    xb     = transpose(x, (1,0,2))            # (B, S, D)
    xm     = xb * (~src_mask)[:, :, None]     # zero padded tokens
    left   [b,g,d] = sum_{s<=g} xm[b,s,d]     # masked prefix sums
    right  [b,g,d] = sum_{s>g}  xm[b,s,d]
    out    = concat([left, right], axis=2)    # (B, G, 2D)

Strategy: pure data parallel over B across 8 cores (64 batches each).
The pad mask is folded into x on the host (exact: multiply by 0/1).

Algebraic output compression (target_regime=memory): the device
computes the full-length masked prefix sums P[g,b,d] = sum_{s<=g} xm
for g = 0..127 as ONE TensorEngine matmul per batch against the
constant 0/1 upper-triangular matrix (contraction over S=128 on
partitions). Row g=G-1=126 and below are `left`; the LAST row g=127 is
the total row-sum, from which the host reconstructs
right = total - left during the gather/unshard step (one broadcasted
f32 subtract). This halves both the device's write traffic and its
matmul/copy work vs materializing `right` on-device.

Memory path:
  * All HBM I/O is bf16: x is pre-masked and rounded on the host (pad
    entries stay exactly 0), tri is exact 0/1, matmul accumulates in
    f32 PSUM, the PSUM->SBUF copy rounds to bf16. ~2e-3 rel err vs the
    2e-2 gate, for half the DMA bytes each way.
  * Writes go through SWDGE (gpsimd). Measured on this stack: a single
    HWDGE dynamic WRITE executes entirely on SDMA engine 0 (~27 GB/s)
    — only HWDGE READS fan out across the 16 engines — while each
    SWDGE DMA binds to one engine (~26 GB/s) with round-robin engine
    assignment across DMAs. So after staging HB batches in SBUF the
    write is split into 16 concurrent SWDGE DMAs, one per group of 8
    output partitions, each descriptor covering a [g, HB, D]
    contiguous DRAM run (1 descriptor per partition per DMA, 16KB).
  * Reads stay on the SP HWDGE ring (16-engine fan-out, ~360 GB/s).
"""

import ml_dtypes
import numpy as np

import concourse.bass as bass
import concourse.mybir as mybir
from concourse import bacc
from concourse.tile import TileContext
from concourse.bass_utils import run_bass_kernel_spmd

S, B, D = 128, 512, 512
G = S - 1
N_CORES = 8
BL = B // N_CORES  # 64 batches per core

IN_CHUNK = 8   # batches per input DMA
# Batches staged per write round. Writes are capped at ~212 GB/s (8
# SWDGE DMAs in flight x ~26.5 GB/s per engine), i.e. ~1.6 staged
# batches/us, while staging produces ~1.8 batches/us — so the write
# pipeline binds end-to-end. Small first rounds start the writes ~13us
# earlier; later rounds are bigger to keep the gpsimd descriptor-gen
# cost (~0.6us per DMA issue) off the critical path.
ROUNDS = [8, 8, 16, 16, 16]
NW = 16        # write DMAs per round (partition groups of 8)

_NC_CACHE = None


def _build_bass(repeats: int = 1) -> bass.Bass:
    """repeats>1 re-runs the whole body (same I/O) — timing calibration only."""
    nc = bacc.Bacc(dynamic_dma_scratch_size=32768)
    f32 = mybir.dt.float32
    bf16 = mybir.dt.bfloat16

    x_in = nc.declare_dram_parameter("x", [S, BL, D], bf16, isOutput=False)
    # tri[s, g] = 1 if s <= g (incl diag): column g holds the g-prefix;
    # column S-1 is all-ones, so PSUM row S-1 is the total row-sum.
    t_in = nc.declare_dram_parameter("tri", [S, S], bf16, isOutput=False)
    # g-major per-core output: partition row g maps to a contiguous
    # (BL, D) DRAM run; row S-1 carries the per-batch totals. The host
    # transposes and reconstructs `right` while gathering.
    out = nc.declare_dram_parameter("out", [S, BL, D], bf16, isOutput=True)

    # partition groups for the write DMAs: 12 SWDGE groups of 8
    # partitions + one 16-partition group per HWDGE ring. SWDGE is
    # capped at 8 in-flight DMAs (~212 GB/s); the HWDGE rings have no
    # such cap and their 16-partition write DMAs fan out one descriptor
    # per SDMA engine, so they keep the engines fed across SWDGE lane
    # turnarounds. (8-partition HWDGE writes land only on engines 64-71,
    # and >16-partition HWDGE writes serialize on one engine — 16 is
    # the sweet spot.)
    sw_groups = [(8 * i, 8 * i + 8) for i in range(12)]

    with TileContext(nc) as tc:
        with (
            tc.tile_pool(name="const", bufs=1) as cpool,
            tc.tile_pool(name="xin", bufs=4) as xpool,
            tc.tile_pool(name="outs", bufs=4) as opool,
            tc.tile_pool(name="psum", bufs=4, space="PSUM") as ppool,
        ):
            trir = cpool.tile([S, S], bf16)
            nc.sync.dma_start(out=trir[:], in_=t_in[:])
            ut = trir[:]  # (128, 128) stationary prefix weights

            def per_pair(xt, ot, j, k):
                """2 batches (j, j+1) of xt -> slots (k, k+1) of ot.

                One 2-bank PSUM tile takes both prefix matmuls; DVE copies
                batch j, ACT copies batch j+1 — one copy op per batch,
                halving the cross-engine semaphore edges vs per-batch tiles.
                """
                ps = ppool.tile([S, 2, D], f32)  # 2 adjacent banks
                for b in range(2):
                    nc.tensor.matmul(out=ps[:, b, :], lhsT=ut,
                                     rhs=xt[:, j + b, :], start=True, stop=True)
                nc.vector.tensor_copy(out=ot[:, k, :], in_=ps[:, 0, :])
                nc.scalar.activation(out=ot[:, k + 1, :], in_=ps[:, 1, :],
                                     func=mybir.ActivationFunctionType.Copy)

            assert sum(ROUNDS) == BL and all(hb % 2 == 0 for hb in ROUNDS)
            starts = [sum(ROUNDS[:r]) for r in range(len(ROUNDS))]
            for _ in range(repeats):
                for h0, hb in zip(starts, ROUNDS):
                    # fixed-size tiles: a pool can't mix tile shapes
                    ot = opool.tile([S, max(ROUNDS), D], bf16)
                    for c0 in range(h0, h0 + hb, min(IN_CHUNK, hb)):
                        ic = min(IN_CHUNK, hb)
                        xt = xpool.tile([S, ic, D], bf16)
                        nc.sync.dma_start(
                            out=xt[:], in_=x_in[:, c0 : c0 + ic, :])
                        for j in range(0, ic, 2):
                            per_pair(xt, ot, j, c0 - h0 + j)
                    for g0, g1 in sw_groups:
                        nc.gpsimd.dma_start(
                            out=out[g0:g1, h0 : h0 + hb, :],
                            in_=ot[g0:g1, 0:hb, :],
                            single_packet=True,
                        )
                    nc.scalar.dma_start(
                        out=out[96:112, h0 : h0 + hb, :],
                        in_=ot[96:112, 0:hb, :],
                    )
                    nc.sync.dma_start(
                        out=out[112:128, h0 : h0 + hb, :],
                        in_=ot[112:128, 0:hb, :],
                    )
    nc.finalize()  # runs the Bacc pass pipeline (reg alloc, wait splitting)
    return nc


def _get_nc() -> bass.Bass:
    global _NC_CACHE
    if _NC_CACHE is None:
        _NC_CACHE = _build_bass()
    return _NC_CACHE


def _make_in_maps(x: np.ndarray, src_mask: np.ndarray) -> list[dict]:
    x = np.asarray(x, dtype=np.float32)
    src_mask = np.asarray(src_mask)
    assert x.shape == (S, B, D), x.shape
    assert src_mask.shape == (B, S), src_mask.shape

    valid = (~src_mask.astype(bool)).astype(np.float32).T  # (S, B)
    # exact zero at padded tokens, then round once to bf16 for the DMA
    xm = (x * valid[:, :, None]).astype(ml_dtypes.bfloat16)
    tri = np.triu(np.ones((S, S), np.float32)).astype(ml_dtypes.bfloat16)

    in_maps = []
    for i in range(N_CORES):
        sl = slice(i * BL, (i + 1) * BL)
        in_maps.append(
            {
                "x": np.ascontiguousarray(xm[:, sl, :]),
                "tri": tri,
            }
        )
    return in_maps


def _assemble(results: list[dict]) -> np.ndarray:
    full = np.empty((B, G, 2 * D), dtype=np.float32)
    for i in range(N_CORES):
        p = np.asarray(results[i]["out"], dtype=np.float32)  # (S, BL, D)
        left = p[0:G].transpose(1, 0, 2)                     # (BL, G, D)
        total = p[G]                                         # (BL, D)
        sl = slice(i * BL, (i + 1) * BL)
        full[sl, :, 0:D] = left
        full[sl, :, D:] = total[:, None, :] - left
    return full


def kernel(x: np.ndarray, src_mask: np.ndarray) -> np.ndarray:
    in_maps = _make_in_maps(x, src_mask)
    res = run_bass_kernel_spmd(_get_nc(), in_maps, core_ids=list(range(N_CORES)))
    return _assemble(res.results)


# revision 20
# speedup vs baseline: 1.0817x; 1.0817x over previous
"""Trainium2 Bass kernel for nn_FragAttention (segment_reduce).

Reference computation (S=128, B=512, D=512, G=S-1=127):
    xb     = transpose(x, (1,0,2))            # (B, S, D)
    xm     = xb * (~src_mask)[:, :, None]     # zero padded tokens
    left   [b,g,d] = sum_{s<=g} xm[b,s,d]     # masked prefix sums
    right  [b,g,d] = sum_{s>g}  xm[b,s,d]
    out    = concat([left, right], axis=2)    # (B, G, 2D)

Strategy: pure data parallel over B across 8 cores (64 batches each).
The pad mask is folded into x on the host (exact: multiply by 0/1).

Algebraic output compression (target_regime=memory): the device
computes the full-length masked prefix sums P[g,b,d] = sum_{s<=g} xm
for g = 0..127 as ONE TensorEngine matmul per batch against the
constant 0/1 upper-triangular matrix (contraction over S=128 on
partitions). Row g=G-1=126 and below are `left`; the LAST row g=127 is
the total row-sum, from which the host reconstructs
right = total - left during the gather/unshard step (one broadcasted
f32 subtract). This halves both the device's write traffic and its
matmul/copy work vs materializing `right` on-device.

Memory path:
  * All HBM I/O is bf16: x is pre-masked and rounded on the host (pad
    entries stay exactly 0), tri is exact 0/1, matmul accumulates in
    f32 PSUM, the PSUM->SBUF copy rounds to bf16. ~2e-3 rel err vs the
    2e-2 gate, for half the DMA bytes each way.
  * Writes go through SWDGE (gpsimd). Measured on this stack: a single
    HWDGE dynamic WRITE executes entirely on SDMA engine 0 (~27 GB/s)
    — only HWDGE READS fan out across the 16 engines — while each
    SWDGE DMA binds to one engine (~26 GB/s) with round-robin engine
    assignment across DMAs. So after staging HB batches in SBUF the
    write is split into 16 concurrent SWDGE DMAs, one per group of 8
    output partitions, each descriptor covering a [g, HB, D]
    contiguous DRAM run (1 descriptor per partition per DMA, 16KB).
  * Reads stay on the SP HWDGE ring (16-engine fan-out, ~360 GB/s).
"""

import ml_dtypes
import numpy as np

import concourse.bass as bass
import concourse.mybir as mybir
from concourse import bacc
from concourse.tile import TileContext
from concourse.bass_utils import run_bass_kernel_spmd

S, B, D = 128, 512, 512
G = S - 1
N_CORES = 8
BL = B // N_CORES  # 64 batches per core

IN_CHUNK = 8   # batches per input DMA
# Batches staged per write round. Writes are capped at ~212 GB/s (8
# SWDGE DMAs in flight x ~26.5 GB/s per engine), i.e. ~1.6 staged
# batches/us, while staging produces ~1.8 batches/us — so the write
# pipeline binds end-to-end. Small first rounds start the writes ~13us
# earlier; later rounds are bigger to keep the gpsimd descriptor-gen
# cost (~0.6us per DMA issue) off the critical path.
ROUNDS = [16, 16, 16, 16]
NW = 16        # write DMAs per round (partition groups of 8)

_NC_CACHE = None


def _build_bass(repeats: int = 1) -> bass.Bass:
    """repeats>1 re-runs the whole body (same I/O) — timing calibration only."""
    nc = bacc.Bacc(dynamic_dma_scratch_size=32768)
    f32 = mybir.dt.float32
    bf16 = mybir.dt.bfloat16

    x_in = nc.declare_dram_parameter("x", [S, BL, D], bf16, isOutput=False)
    # tri[s, g] = 1 if s <= g (incl diag): column g holds the g-prefix;
    # column S-1 is all-ones, so PSUM row S-1 is the total row-sum.
    t_in = nc.declare_dram_parameter("tri", [S, S], bf16, isOutput=False)
    # g-major per-core output: partition row g maps to a contiguous
    # (BL, D) DRAM run; row S-1 carries the per-batch totals. The host
    # transposes and reconstructs `right` while gathering.
    out = nc.declare_dram_parameter("out", [S, BL, D], bf16, isOutput=True)

    # partition groups for the write DMAs: 12 SWDGE groups of 8
    # partitions + one 16-partition group per HWDGE ring. SWDGE is
    # capped at 8 in-flight DMAs (~212 GB/s); the HWDGE rings have no
    # such cap and their 16-partition write DMAs fan out one descriptor
    # per SDMA engine, so they keep the engines fed across SWDGE lane
    # turnarounds. (8-partition HWDGE writes land only on engines 64-71,
    # and >16-partition HWDGE writes serialize on one engine — 16 is
    # the sweet spot.)
    sw_groups = [(8 * i, 8 * i + 8) for i in range(12)]

    with TileContext(nc) as tc:
        with (
            tc.tile_pool(name="const", bufs=1) as cpool,
            tc.tile_pool(name="xin", bufs=4) as xpool,
            tc.tile_pool(name="outs", bufs=4) as opool,
            tc.tile_pool(name="psum", bufs=4, space="PSUM") as ppool,
        ):
            trir = cpool.tile([S, S], bf16)
            nc.sync.dma_start(out=trir[:], in_=t_in[:])
            ut = trir[:]  # (128, 128) stationary prefix weights

            def per_pair(xt, ot, j, k):
                """2 batches (j, j+1) of xt -> slots (k, k+1) of ot.

                One 2-bank PSUM tile takes both prefix matmuls; DVE copies
                batch j, ACT copies batch j+1 — one copy op per batch,
                halving the cross-engine semaphore edges vs per-batch tiles.
                """
                ps = ppool.tile([S, 2, D], f32)  # 2 adjacent banks
                for b in range(2):
                    nc.tensor.matmul(out=ps[:, b, :], lhsT=ut,
                                     rhs=xt[:, j + b, :], start=True, stop=True)
                nc.vector.tensor_copy(out=ot[:, k, :], in_=ps[:, 0, :])
                nc.scalar.activation(out=ot[:, k + 1, :], in_=ps[:, 1, :],
                                     func=mybir.ActivationFunctionType.Copy)

            assert sum(ROUNDS) == BL and all(hb % 2 == 0 for hb in ROUNDS)
            starts = [sum(ROUNDS[:r]) for r in range(len(ROUNDS))]
            for _ in range(repeats):
                for h0, hb in zip(starts, ROUNDS):
                    # fixed-size tiles: a pool can't mix tile shapes
                    ot = opool.tile([S, max(ROUNDS), D], bf16)
                    for c0 in range(h0, h0 + hb, min(IN_CHUNK, hb)):
                        ic = min(IN_CHUNK, hb)
                        xt = xpool.tile([S, ic, D], bf16)
                        nc.sync.dma_start(
                            out=xt[:], in_=x_in[:, c0 : c0 + ic, :])
                        for j in range(0, ic, 2):
                            per_pair(xt, ot, j, c0 - h0 + j)
                    for g0, g1 in sw_groups:
                        nc.gpsimd.dma_start(
                            out=out[g0:g1, h0 : h0 + hb, :],
                            in_=ot[g0:g1, 0:hb, :],
                            single_packet=True,
                        )
                    nc.scalar.dma_start(
                        out=out[96:112, h0 : h0 + hb, :],
                        in_=ot[96:112, 0:hb, :],
                    )
                    nc.sync.dma_start(
                        out=out[112:128, h0 : h0 + hb, :],
                        in_=ot[112:128, 0:hb, :],
                    )
    nc.finalize()  # runs the Bacc pass pipeline (reg alloc, wait splitting)
    return nc


def _get_nc() -> bass.Bass:
    global _NC_CACHE
    if _NC_CACHE is None:
        _NC_CACHE = _build_bass()
    return _NC_CACHE


def _make_in_maps(x: np.ndarray, src_mask: np.ndarray) -> list[dict]:
    x = np.asarray(x, dtype=np.float32)
    src_mask = np.asarray(src_mask)
    assert x.shape == (S, B, D), x.shape
    assert src_mask.shape == (B, S), src_mask.shape

    valid = (~src_mask.astype(bool)).astype(np.float32).T  # (S, B)
    # exact zero at padded tokens, then round once to bf16 for the DMA
    xm = (x * valid[:, :, None]).astype(ml_dtypes.bfloat16)
    tri = np.triu(np.ones((S, S), np.float32)).astype(ml_dtypes.bfloat16)

    in_maps = []
    for i in range(N_CORES):
        sl = slice(i * BL, (i + 1) * BL)
        in_maps.append(
            {
                "x": np.ascontiguousarray(xm[:, sl, :]),
                "tri": tri,
            }
        )
    return in_maps


def _assemble(results: list[dict]) -> np.ndarray:
    full = np.empty((B, G, 2 * D), dtype=np.float32)
    for i in range(N_CORES):
        p = np.asarray(results[i]["out"], dtype=np.float32)  # (S, BL, D)
        left = p[0:G].transpose(1, 0, 2)                     # (BL, G, D)
        total = p[G]                                         # (BL, D)
        sl = slice(i * BL, (i + 1) * BL)
        full[sl, :, 0:D] = left
        full[sl, :, D:] = total[:, None, :] - left
    return full


def kernel(x: np.ndarray, src_mask: np.ndarray) -> np.ndarray:
    in_maps = _make_in_maps(x, src_mask)
    res = run_bass_kernel_spmd(_get_nc(), in_maps, core_ids=list(range(N_CORES)))
    return _assemble(res.results)
